# revision 5
# baseline (speedup 1.0000x reference)
"""Trainium2 Bass kernel for the DeltaSynapse message-passing einsum.

Computes  I[b,o] = einsum('eo,dbe,deo,dbe->bo', signs*W, Xd, delaymap, Wshort)
with D=8, B=16, E=4096, O=4096, fp32.

Strategy (tensor-parallel over the post dim o, 8 cores, no collectives):
  - Each core owns a 512-wide o-shard of the output.
  - Host-side input prep folds the small elementwise factors:
      Weff = signs*W                      [E, O]   (bf16)
      A    = Xd*Wshort -> A_T [E, D*B]             (bf16)
      Md   = delaymap * Weff              [D, E, O] (bf16)  <- the big stream
    bf16 keeps rel err ~1e-3 (well under the 2e-2 gate) while halving the
    dominant HBM traffic vs fp32 and removing all per-element DVE work.
  - Per e-chunk of 512 rows the kernel streams the K_DMA premultiplied
    delay planes (split across both HWDGE rings) and feeds the PE with
    accumulating bf16 matmuls (contract e, out [16,512] in one PSUM bank).
  - Hybrid mode (K_DMA < D): the remaining planes are synthesized on-chip
    from dsel (=argmax_d delaymap, bf16) and Weff via DVE
    tensor_scalar(is_equal) + tensor_mul, trading DMA bytes for DVE cycles.
"""

import sys

import numpy as np

sys.path.insert(0, "/opt/trn_rl_repo")

import ml_dtypes

BF16 = ml_dtypes.bfloat16

D, B, E, O = 8, 16, 4096, 4096
NCORES = 8
OS = O // NCORES        # 512: per-core o width
CH = 512                # e-rows per chunk
NCH = E // CH           # 8 chunks
RP = CH // 128          # 4 e-rows per SBUF partition
DB = D * B              # 128

# Number of delay planes streamed as host-premultiplied bf16 Md; the other
# D-K_DMA planes are built on-chip from (dsel, Weff) on the DVE.
K_DMA = 8

_CACHE = {}


def build_nc(k_dma=None):
    import concourse.mybir as mybir
    from concourse import bacc
    from concourse.tile import TileContext

    if k_dma is None:
        k_dma = K_DMA
    f32 = mybir.dt.float32
    bf16 = mybir.dt.bfloat16
    n_dve = D - k_dma           # planes synthesized on-chip

    nc = bacc.Bacc()
    md = None
    dsel = None
    weff = None
    if k_dma:
        # host-prepermuted to the SBUF tile layout: chunk-major, partition,
        # then (d, r, o) — every per-chunk DMA is fully contiguous.
        md = nc.dram_tensor(
            "md", [NCH, 128, k_dma * RP * OS], bf16, kind="ExternalInput")
    if n_dve:
        dsel = nc.dram_tensor("dsel", [E, OS], bf16, kind="ExternalInput")
        weff = nc.dram_tensor("weff", [E, OS], bf16, kind="ExternalInput")
    at = nc.dram_tensor("at", [E, DB], bf16, kind="ExternalInput")
    out = nc.dram_tensor("out", [B, OS], f32, kind="ExternalOutput")

    with TileContext(nc) as tc:
        with (
            tc.tile_pool(name="mdp", bufs=3) as md_pool,
            tc.tile_pool(name="selp", bufs=2) as sel_pool,
            tc.tile_pool(name="mvp", bufs=4) as mv_pool,
            tc.tile_pool(name="atp", bufs=1) as at_pool,
            tc.tile_pool(name="outp", bufs=1) as out_pool,
            tc.tile_pool(name="ps", bufs=1, space="PSUM") as psum_pool,
        ):
            # A_T = (Xd*Wshort) transposed to [e, d*B+b]; e on partitions.
            # Within chunk c, e(p, j) = c*CH + RP*p + j, matching the Md tiles.
            at_p = at_pool.tile([128, NCH * RP * DB], bf16, tag="atp")

            psum_t = psum_pool.tile([B, OS], f32)
            n_mm = NCH * D * RP
            mm = 0
            for c in range(NCH):
                es = slice(c * CH, (c + 1) * CH)
                nc.sync.dma_start(
                    out=at_p[:, c * RP * DB:(c + 1) * RP * DB],
                    in_=at[es, :].rearrange("(p r) k -> p (r k)", p=128))

                m_t = None
                if k_dma:
                    # premultiplied planes, byte-balanced across both rings
                    m_t = md_pool.tile([128, k_dma * RP * OS], bf16, tag="md")
                    half = (k_dma + 1) // 2 * RP * OS
                    nc.scalar.dma_start(
                        out=m_t[:, :half], in_=md[c, :, :half])
                    if k_dma * RP * OS > half:
                        nc.sync.dma_start(
                            out=m_t[:, half:], in_=md[c, :, half:])

                dsel_t = None
                weff_t = None
                if n_dve:
                    dsel_t = sel_pool.tile([128, RP * OS], bf16, tag="dsel")
                    weff_t = sel_pool.tile([128, RP * OS], bf16, tag="weff")
                    nc.sync.dma_start(
                        out=dsel_t,
                        in_=dsel[es, :].rearrange("(p r) o -> p (r o)", p=128))
                    nc.sync.dma_start(
                        out=weff_t,
                        in_=weff[es, :].rearrange("(p r) o -> p (r o)", p=128))

                for d in range(D):
                    if d < k_dma:
                        plane = m_t[:, d * RP * OS:(d + 1) * RP * OS]
                    else:
                        mask_t = mv_pool.tile([128, RP * OS], bf16, tag="mask")
                        nc.vector.tensor_scalar(
                            mask_t, dsel_t, float(d), None,
                            mybir.AluOpType.is_equal)
                        mv_t = mv_pool.tile([128, RP * OS], bf16, tag="mv")
                        nc.vector.tensor_mul(mv_t, mask_t, weff_t)
                        plane = mv_t
                    for j in range(RP):
                        lhsT = at_p[:, c * RP * DB + j * DB + d * B:
                                    c * RP * DB + j * DB + d * B + B]
                        rhs = plane[:, j * OS:(j + 1) * OS]
                        nc.tensor.matmul(
                            psum_t, lhsT=lhsT, rhs=rhs,
                            start=(mm == 0), stop=(mm == n_mm - 1))
                        mm += 1

            out_t = out_pool.tile([B, OS], f32)
            nc.vector.tensor_copy(out_t, psum_t)
            nc.sync.dma_start(out=out[:, :], in_=out_t)

    nc.finalize()
    return nc


def _get_nc():
    if "nc" not in _CACHE:
        _CACHE["nc"] = build_nc()
    return _CACHE["nc"]


def prepare_in_maps(W, signs, Xd, delaymap, Wshort, k_dma=None):
    if k_dma is None:
        k_dma = K_DMA
    W = np.asarray(W, dtype=np.float32)
    signs = np.asarray(signs, dtype=np.float32)
    Xd = np.asarray(Xd, dtype=np.float32)
    delaymap = np.asarray(delaymap, dtype=np.float32)
    Wshort = np.asarray(Wshort, dtype=np.float32)

    weff = signs * W                                   # [E, O] f32
    a = Xd * Wshort                                    # [D, B, E]
    at = np.ascontiguousarray(
        a.transpose(2, 0, 1).reshape(E, DB)).astype(BF16)
    n_dve = D - k_dma
    if n_dve:
        dsel_full = np.argmax(delaymap, axis=0).astype(BF16)   # [E, O]

    in_maps = []
    for m in range(NCORES):
        sl = slice(m * OS, (m + 1) * OS)
        im = {"at": at}
        if k_dma:
            md_m = (delaymap[:k_dma, :, sl] * weff[None, :, sl]).astype(BF16)
            # [k, E, OS] -> [NCH, 128, k*RP*OS] (chunk, partition, (d r o))
            im["md"] = np.ascontiguousarray(
                md_m.reshape(k_dma, NCH, 128, RP, OS)
                .transpose(1, 2, 0, 3, 4)
                .reshape(NCH, 128, k_dma * RP * OS))
        if n_dve:
            im["dsel"] = np.ascontiguousarray(dsel_full[:, sl])
            im["weff"] = np.ascontiguousarray(weff[:, sl].astype(BF16))
        in_maps.append(im)
    return in_maps


def kernel(W, signs, Xd, delaymap, Wshort):
    from concourse.bass_utils import run_bass_kernel_spmd

    in_maps = prepare_in_maps(W, signs, Xd, delaymap, Wshort)
    nc = _get_nc()
    res = run_bass_kernel_spmd(nc, in_maps, core_ids=list(range(NCORES)))
    return np.concatenate([r["out"] for r in res.results], axis=1)
